# revision 3
# baseline (speedup 1.0000x reference)
"""Causal multi-head attention with RoPE on 8 Trainium2 NeuronCores.

Problem: B=2, N=2048, DIM=1024, H=16, DH=64, fp32 in/out.
Sharding: head-parallel — core c owns heads 2c, 2c+1 (columns c*128:(c+1)*128
of Wq/Wk/Wv, rows c*128:(c+1)*128 of Wo) for both batches. Each core computes
its partial output projection [DIM, B*N]; the host sums the 8 partials
(the "all-reduce") and adds the bias.

All matmul operands are bf16 (fp32r at full PE rate draws enough power to
trigger sustained DVS throttling — util capped near 50% — so bf16 is ~2x
faster in practice and halves LDWEIGHTS/SBUF/DMA traffic). PSUM accumulation
stays fp32; tolerance is 2e-2 so bf16 operands are comfortably accurate.

On-device pipeline:
  p1(chunk): QKV projections (K=1024 accum in PSUM), RoPE applied from PSUM
             via DVE in bf16 -> qT/kT resident [128, 4096]; V transposed via
             PE into [j, d] layout, augmented with a 64-col ones BLOCK.
  p2(b, ch): per head: S^T = k q^T per 128-j-block with the causal mask
             added in-PSUM via an identity-x-mask matmul; exp on ACT ->
             bf16; augmented V-matmul accumulates O'^T (rows 0:64) and the
             softmax sum replicated across rows 64:128 (ones block) -> the
             normalize is a plain [64,512] reciprocal + mul on DVE, no
             broadcast; fused Wo projection per token chunk.
Emission order interleaves batch-1 p1 with batch-0 p2 so DVE-heavy RoPE
overlaps PE-heavy attention. Single PSUM pool (8 banks) open throughout.
"""
import numpy as np
import ml_dtypes
import bass_rust
from concourse import bacc
import concourse.mybir as mybir
from concourse.tile import TileContext
from concourse.bass_utils import run_bass_kernel_spmd

B, N, DIM, H, DH = 2, 2048, 1024, 16, 64
NCORES = 8
HPC = H // NCORES          # 2 heads per core
T = B * N                  # 4096 tokens
CHUNK = 512
NCH = T // CHUNK           # 8 token chunks
NCB = DIM // 128           # 8 contraction blocks
NINST = B * HPC            # 4 attention instances per core
NJB = N // 128             # 16 j-blocks per batch
NEG = -1e9

F32 = mybir.dt.float32
BF16 = mybir.dt.bfloat16
NPBF16 = ml_dtypes.bfloat16

_NC_CACHE = {}


def build(reps=1):
    nc = bacc.Bacc()
    xTD = nc.dram_tensor("xT", [DIM, T], BF16, kind="ExternalInput")
    wqD = nc.dram_tensor("wq", [DIM, 128], BF16, kind="ExternalInput")
    wkD = nc.dram_tensor("wk", [DIM, 128], BF16, kind="ExternalInput")
    wvD = nc.dram_tensor("wv", [DIM, 128], BF16, kind="ExternalInput")
    woD = nc.dram_tensor("wo", [128, DIM], BF16, kind="ExternalInput")
    cosD = nc.dram_tensor("cosT", [DH, N], BF16, kind="ExternalInput")
    sinsD = nc.dram_tensor("sinsT", [DH, N], BF16, kind="ExternalInput")
    identD = nc.dram_tensor("identD", [128, 128], BF16, kind="ExternalInput")
    ident2D = nc.dram_tensor("ident2D", [128, DH], BF16, kind="ExternalInput")
    masksD = nc.dram_tensor("masksD", [128, 4, CHUNK], BF16, kind="ExternalInput")
    onesBlkD = nc.dram_tensor("onesBlkD", [128, NINST, NJB, DH], BF16,
                              kind="ExternalInput")
    outD = nc.dram_tensor("outT", [DIM, T], BF16, kind="ExternalOutput")

    Exp = mybir.ActivationFunctionType.Exp

    with TileContext(nc) as tc:
        with (
            tc.tile_pool(name="const", bufs=1) as cp,
            tc.tile_pool(name="sb", bufs=2) as sb,
            tc.tile_pool(name="ps", bufs=1, space="PSUM") as ps,
        ):
            ident = cp.tile([128, 128], BF16)
            ident2 = cp.tile([128, DH], BF16)
            masks = cp.tile([128, 4, CHUNK], BF16)
            wq = cp.tile([128, NCB, 128], BF16)
            wk = cp.tile([128, NCB, 128], BF16)
            wv = cp.tile([128, NCB, 128], BF16)
            wo = cp.tile([128, NCB, 128], BF16)
            cos2 = cp.tile([128, T], BF16)
            sins2 = cp.tile([128, T], BF16)
            qt = cp.tile([128, T], BF16)     # rows 0:64 head0, 64:128 head1
            kt = cp.tile([128, T], BF16)
            # per (inst, jb): cols 0:64 = V^T block, cols 64:128 = ones
            v_aug = cp.tile([128, NINST, NJB, 2 * DH], BF16)

            # minimal front-load: wq, then chunk-0 x, then the rest
            nc.sync.dma_start(
                out=wq, in_=wqD[:].rearrange("(cb p) d -> p cb d", p=128))
            xt0 = sb.tile([128, NCB, CHUNK], BF16, tag="xt", bufs=2,
                          name="xt_first")
            for hb in range(2):
                nc.sync.dma_start(
                    out=xt0[:, hb * 4:(hb + 1) * 4, :],
                    in_=xTD[hb * 4 * 128:(hb + 1) * 4 * 128, 0:CHUNK].rearrange(
                        "(cb p) n -> p cb n", p=128))
            for t, d in ((wk, wkD), (wv, wvD)):
                nc.sync.dma_start(
                    out=t, in_=d[:].rearrange("(cb p) d -> p cb d", p=128))
            nc.sync.dma_start(out=cos2[0:64, 0:N], in_=cosD[:])
            nc.sync.dma_start(out=sins2[0:64, 0:N], in_=sinsD[:])
            nc.sync.dma_start(out=ident2, in_=ident2D[:])
            # on-device duplication: rows first (chunk 0 needs them)
            for t in (cos2, sins2):
                nc.sync.dma_start(out=t[64:128, 0:N], in_=t[0:64, 0:N])

            # deferred constants (p2/transposes only) ride the SWDGE queues
            nc.gpsimd.dma_start(out=v_aug[:, :, :, DH:2 * DH], in_=onesBlkD[:])
            nc.gpsimd.dma_start(out=ident, in_=identD[:])
            nc.gpsimd.dma_start(out=masks, in_=masksD[:])
            nc.gpsimd.dma_start(
                out=wo, in_=woD[:].rearrange("p (db d) -> p db d", d=128))

            def dup_cossin_cols():
                for t in (cos2, sins2):
                    nc.gpsimd.dma_start(out=t[0:64, N:2 * N], in_=t[0:64, 0:N])
                    nc.gpsimd.dma_start(out=t[64:128, N:2 * N], in_=t[0:64, 0:N])

            def p1_chunk(ch):
                """QKV + RoPE + V transpose for one 512-token chunk."""
                t0 = ch * CHUNK
                if ch == 0:
                    xt = xt0
                else:
                    xt = sb.tile([128, NCB, CHUNK], BF16, tag="xt", bufs=2,
                                 name=f"xt{ch}")
                    half = NCB // 2
                    for hb in range(2):
                        nc.sync.dma_start(
                            out=xt[:, hb * half:(hb + 1) * half, :],
                            in_=xTD[hb * half * 128:(hb + 1) * half * 128,
                                    t0:t0 + CHUNK].rearrange(
                                "(cb p) n -> p cb n", p=128))
                csl = slice(t0, t0 + CHUNK)
                for which, W in (("q", wq), ("k", wk), ("v", wv)):
                    pp = ps.tile([128, 2, CHUNK], F32, tag="big", bufs=2,
                                 name=f"pp{ch}{which}")
                    for cb in range(NCB):
                        nc.tensor.matmul(pp[:, 0, :], W[:, cb, :], xt[:, cb, :],
                                         start=(cb == 0), stop=(cb == NCB - 1))
                    if which in ("q", "k"):
                        dst = qt if which == "q" else kt
                        raw = sb.tile([128, CHUNK], BF16, tag="raw", bufs=3,
                                      name=f"w{ch}{which}")
                        nc.vector.tensor_copy(raw, pp[:, 0, :])
                        rawsw = sb.tile([128, CHUNK], BF16, tag="rawsw", bufs=3,
                                        name=f"x{ch}{which}")
                        for hh in (0, 64):
                            a, bnd, c2 = hh, hh + 32, hh + 64
                            nc.gpsimd.dma_start(out=rawsw[a:bnd, :],
                                                in_=raw[bnd:c2, :])
                            nc.gpsimd.dma_start(out=rawsw[bnd:c2, :],
                                                in_=raw[a:bnd, :])
                        tmp = sb.tile([128, CHUNK], BF16, tag="tmp", bufs=2,
                                      name=f"t{ch}{which}")
                        tmp2 = sb.tile([128, CHUNK], BF16, tag="tmp2", bufs=2,
                                       name=f"u{ch}{which}")
                        nc.vector.tensor_mul(tmp, raw, cos2[:, csl])
                        nc.vector.tensor_mul(tmp2, rawsw, sins2[:, csl])
                        nc.vector.tensor_add(dst[:, csl], tmp, tmp2)
                    else:
                        vtc = sb.tile([128, CHUNK], BF16, tag="vtc", bufs=2,
                                      name=f"v{ch}")
                        nc.vector.tensor_copy(vtc, pp[:, 0, :])
                        bidx = ch // 4
                        for tb in range(4):
                            jb = (ch % 4) * 4 + tb
                            for h in range(HPC):
                                pt = ps.tile([128, DH], BF16, tag="sm",
                                             bufs=2, name=f"pt{ch}{tb}{h}")
                                nc.tensor.transpose(
                                    pt,
                                    vtc[h * 64:(h + 1) * 64,
                                        tb * 128:(tb + 1) * 128],
                                    ident2[h * 64:(h + 1) * 64, :])
                                nc.vector.tensor_copy(
                                    v_aug[:, bidx * HPC + h, jb, 0:DH], pt)

            def p2_chunk(bidx, ch):
                """Attention + projection for i-chunk ch of batch bidx."""
                gcol = bidx * N + ch * CHUNK
                njb = 4 * (ch + 1)
                ot = sb.tile([128, CHUNK], BF16, tag="ot", bufs=2,
                             name=f"ot{bidx}{ch}")
                for h in range(HPC):
                    inst = bidx * HPC + h
                    qr = slice(h * 64, (h + 1) * 64)
                    po = ps.tile([128, CHUNK], F32, tag="po", bufs=2,
                                 name=f"po{bidx}{ch}{h}")
                    for r0 in range(0, njb, 2):
                        pst = ps.tile([128, 2, CHUNK], F32, tag="big", bufs=2,
                                      name=f"ps{bidx}{ch}{h}{r0}")
                        for idx in range(2):
                            jb = r0 + idx
                            jc = bidx * N + jb * 128
                            diag = jb >= 4 * ch
                            nc.tensor.matmul(
                                pst[:, idx, :], kt[qr, jc:jc + 128],
                                qt[qr, gcol:gcol + CHUNK],
                                start=True, stop=not diag)
                            if diag:
                                nc.tensor.matmul(
                                    pst[:, idx, :], ident,
                                    masks[:, jb - 4 * ch, :],
                                    start=False, stop=True)
                        expt = sb.tile([128, 2, CHUNK], BF16, tag="expt",
                                       bufs=4, name=f"e{bidx}{ch}{h}{r0}")
                        nc.scalar.activation(expt, pst, Exp)
                        for idx in range(2):
                            jb = r0 + idx
                            nc.tensor.matmul(
                                po, v_aug[:, inst, jb, :], expt[:, idx, :],
                                start=(jb == 0), stop=(jb == njb - 1))
                    # normalize: sums are replicated in po rows 64:128
                    rinv = sb.tile([DH, CHUNK], F32, tag="rinv", bufs=2,
                                   name=f"r{bidx}{ch}{h}")
                    nc.vector.reciprocal(rinv, po[DH:2 * DH, :])
                    nc.vector.tensor_mul(ot[qr, :], po[0:DH, :], rinv)
                # fused output projection for this token chunk
                for db in range(NCB):
                    ppr = ps.tile([128, CHUNK], F32, tag="sm", bufs=2,
                                  name=f"pj{bidx}{ch}{db}")
                    nc.tensor.matmul(ppr, wo[:, db, :], ot,
                                     start=True, stop=True)
                    osb = sb.tile([128, CHUNK], BF16, tag="osb", bufs=3,
                                  name=f"o{bidx}{ch}{db}")
                    nc.vector.tensor_copy(osb, ppr)
                    nc.sync.dma_start(
                        out=outD[db * 128:(db + 1) * 128, gcol:gcol + CHUNK],
                        in_=osb)

            for _ in range(reps):
                p1_chunk(0)
                p1_chunk(1)
                p2_chunk(0, 0)
                p1_chunk(2)
                dup_cossin_cols()
                p2_chunk(0, 1)
                p1_chunk(3)
                p2_chunk(0, 2)
                p1_chunk(4)
                p2_chunk(0, 3)
                p1_chunk(5)
                p1_chunk(6)
                p1_chunk(7)
                for ch in (3, 2, 1, 0):
                    p2_chunk(1, ch)
    nc.compile()
    return nc


def _get_nc(reps=1):
    if reps not in _NC_CACHE:
        _NC_CACHE[reps] = build(reps)
    return _NC_CACHE[reps]


def make_in_maps(x, pos_emb, Wq, Wk, Wv, Wo):
    x = np.asarray(x, np.float32)
    pos_emb = np.asarray(pos_emb, np.float32)
    Wq = np.asarray(Wq, np.float32)
    Wk = np.asarray(Wk, np.float32)
    Wv = np.asarray(Wv, np.float32)
    Wo = np.asarray(Wo, np.float32)

    xT = np.ascontiguousarray(x.reshape(T, DIM).T).astype(NPBF16)  # [DIM, T]
    cosT = np.ascontiguousarray(np.cos(pos_emb).T).astype(NPBF16)  # [DH, N]
    sinT = np.sin(pos_emb).T
    sinsT = np.ascontiguousarray(
        np.concatenate([-sinT[0:32], sinT[32:64]], axis=0)).astype(NPBF16)
    scale = np.float32(DH ** -0.5)

    ident = np.eye(128, dtype=NPBF16)
    ident2 = np.tile(np.eye(DH, dtype=np.float32), (2, 1)).astype(NPBF16)
    jj = np.arange(128)[:, None]
    ii = np.arange(CHUNK)[None, :]
    masks = np.zeros((128, 4, CHUNK), np.float32)
    for r in range(4):
        masks[:, r, :] = np.where(r * 128 + jj <= ii, 0.0, NEG)
    masks = masks.astype(NPBF16)
    ones_blk = np.ones((128, NINST, NJB, DH), NPBF16)

    in_maps = []
    for c in range(NCORES):
        cols = slice(c * 128, (c + 1) * 128)
        in_maps.append(dict(
            xT=xT,
            wq=(np.ascontiguousarray(Wq[:, cols]) * scale).astype(NPBF16),
            wk=np.ascontiguousarray(Wk[:, cols]).astype(NPBF16),
            wv=np.ascontiguousarray(Wv[:, cols]).astype(NPBF16),
            wo=np.ascontiguousarray(Wo[cols, :]).astype(NPBF16),
            cosT=cosT, sinsT=sinsT, identD=ident, ident2D=ident2,
            masksD=masks, onesBlkD=ones_blk,
        ))
    return in_maps


def run(in_maps, trace=False, reps=1, **kw):
    nc = _get_nc(reps)
    return run_bass_kernel_spmd(nc, in_maps, list(range(NCORES)),
                                trace=trace, **kw)


def kernel(x, pos_emb, Wq, Wk, Wv, Wo, bo):
    in_maps = make_in_maps(x, pos_emb, Wq, Wk, Wv, Wo)
    res = run(in_maps)
    acc = np.zeros((DIM, T), np.float64)
    for c in range(NCORES):
        acc += np.asarray(res.results[c]["outT"], np.float32)
    out = acc.T.reshape(B, N, DIM) + np.asarray(bo, np.float32)[None, None, :]
    return out.astype(np.float32)


# revision 43
# speedup vs baseline: 1.1654x; 1.1654x over previous
"""Causal multi-head attention with RoPE on 8 Trainium2 NeuronCores.

Problem: B=2, N=2048, DIM=1024, H=16, DH=64, fp32 in/out.
Sharding: head-parallel — core c owns heads 2c, 2c+1 (columns c*128:(c+1)*128
of Wq/Wk/Wv, rows c*128:(c+1)*128 of Wo) for both batches. Each core computes
its partial output projection [DIM, B*N]; the host sums the 8 partials
(the "all-reduce") and adds the bias.

All matmul operands are bf16 (fp32r at full PE rate draws enough power to
trigger sustained DVS throttling — util capped near 50% — so bf16 is ~2x
faster in practice and halves LDWEIGHTS/SBUF/DMA traffic). PSUM accumulation
stays fp32; tolerance is 2e-2 so bf16 operands are comfortably accurate.

On-device pipeline:
  p1(chunk): QKV projections (K=1024 accum in PSUM), RoPE applied from PSUM
             via DVE in bf16 -> qT/kT resident [128, 4096]; V transposed via
             PE ([128,128] per token block, both heads at once) into [j, d]
             layout, augmented with a 64-col ones BLOCK.
  p2(b, ch): per head: S^T = k q^T per 128-j-block, causal-tight (columns
             below the diagonal are never computed; the in-block triangle is
             masked in-PSUM via an identity-x-mask matmul on the 128-col
             diagonal stripe only); exp on ACT -> bf16; augmented V-matmul
             accumulates O'^T (rows 0:64) and the softmax sum replicated
             across rows 64:128 (ones block); normalize computes 1/s with an
             integer-magic seed + one Newton step on DVE (~2.5x cheaper than
             InstReciprocal); fused Wo projection per token chunk, with the
             PSUM->SBUF output copies split between DVE and ACT.
Emission interleaves p1 chunks with p2 so DVE-heavy RoPE overlaps PE-heavy
attention, keeping ~1.5 chunks of lookahead (tighter coupling measurably
stalls the PE on RoPE latency). All x chunks are prefetched (bufs=8); input
DMAs are spread across the sync/scalar/gpsimd queues so x loads are never
stuck behind constants. Single PSUM pool (8 banks): pst 4, po 2, sm 2.
"""
import numpy as np
import ml_dtypes
import bass_rust
from concourse import bacc
import concourse.mybir as mybir
from concourse.tile import TileContext
from concourse.bass_utils import run_bass_kernel_spmd

B, N, DIM, H, DH = 2, 2048, 1024, 16, 64
NCORES = 8
HPC = H // NCORES          # 2 heads per core
T = B * N                  # 4096 tokens
CHUNK = 512
NCH = T // CHUNK           # 8 token chunks
NCB = DIM // 128           # 8 contraction blocks
NINST = B * HPC            # 4 attention instances per core
NJB = N // 128             # 16 j-blocks per batch
NEG = -1e9

F32 = mybir.dt.float32
I32 = mybir.dt.int32
BF16 = mybir.dt.bfloat16
NPBF16 = ml_dtypes.bfloat16

_NC_CACHE = {}


def build(reps=1):
    nc = bacc.Bacc()
    xTD = nc.dram_tensor("xT", [DIM, T], BF16, kind="ExternalInput")
    wqD = nc.dram_tensor("wq", [DIM, 128], BF16, kind="ExternalInput")
    wkD = nc.dram_tensor("wk", [DIM, 128], BF16, kind="ExternalInput")
    wvD = nc.dram_tensor("wv", [DIM, 128], BF16, kind="ExternalInput")
    woD = nc.dram_tensor("wo", [128, DIM], BF16, kind="ExternalInput")
    cosD = nc.dram_tensor("cosT", [2 * DH, N], BF16, kind="ExternalInput")
    sinsD = nc.dram_tensor("sinsT", [2 * DH, N], BF16, kind="ExternalInput")
    identD = nc.dram_tensor("identD", [128, 128], BF16, kind="ExternalInput")
    masksD = nc.dram_tensor("masksD", [128, 128], BF16, kind="ExternalInput")
    onesBlkD = nc.dram_tensor("onesBlkD", [128, NINST, NJB, DH], BF16,
                              kind="ExternalInput")
    outD = nc.dram_tensor("outT", [DIM, T], BF16, kind="ExternalOutput")

    Exp = mybir.ActivationFunctionType.Exp

    with TileContext(nc) as tc:
        with (
            tc.tile_pool(name="const", bufs=1) as cp,
            tc.tile_pool(name="sb", bufs=2) as sb,
            tc.tile_pool(name="ps", bufs=1, space="PSUM") as ps,
        ):
            ident = cp.tile([128, 128], BF16)
            masks = cp.tile([128, 128], BF16)
            wq = cp.tile([128, NCB, 128], BF16)
            wk = cp.tile([128, NCB, 128], BF16)
            wv = cp.tile([128, NCB, 128], BF16)
            wo = cp.tile([128, NCB, 128], BF16)
            cos2 = cp.tile([128, T], BF16)
            sins2 = cp.tile([128, T], BF16)
            qt = cp.tile([128, T], BF16)     # rows 0:64 head0, 64:128 head1
            kt = cp.tile([128, T], BF16)
            # per (inst, jb): cols 0:64 = V^T block, cols 64:128 = ones
            v_aug = cp.tile([128, NINST, NJB, 2 * DH], BF16)

            # minimal front-load: wq, then chunk-0 x, then the rest
            nc.sync.dma_start(
                out=wq, in_=wqD[:].rearrange("(cb p) d -> p cb d", p=128))
            xt0 = sb.tile([128, NCB, CHUNK], BF16, tag="xt", bufs=8,
                          name="xt_first")
            nc.sync.dma_start(out=xt0[:, 0:1, :],
                              in_=xTD[0:128, 0:CHUNK].rearrange(
                                  "(cb p) n -> p cb n", p=128))
            nc.sync.dma_start(out=xt0[:, 1:4, :],
                              in_=xTD[128:512, 0:CHUNK].rearrange(
                                  "(cb p) n -> p cb n", p=128))
            nc.sync.dma_start(out=xt0[:, 4:8, :],
                              in_=xTD[512:1024, 0:CHUNK].rearrange(
                                  "(cb p) n -> p cb n", p=128))
            # chunk-0 slice of cos/sin first (tiny, unblocks RoPE), then
            # weights, then the rest of cos/sin
            nc.scalar.dma_start(out=cos2[:, 0:CHUNK], in_=cosD[:, 0:CHUNK])
            nc.scalar.dma_start(out=sins2[:, 0:CHUNK], in_=sinsD[:, 0:CHUNK])
            for t, d in ((wk, wkD), (wv, wvD)):
                nc.scalar.dma_start(
                    out=t, in_=d[:].rearrange("(cb p) d -> p cb d", p=128))
            nc.scalar.dma_start(out=cos2[:, CHUNK:N], in_=cosD[:, CHUNK:N])
            nc.scalar.dma_start(out=sins2[:, CHUNK:N], in_=sinsD[:, CHUNK:N])

            # deferred constants (p2/transposes only) ride the SWDGE queues;
            # small tiles first (chunk-0 transposes need ident, first S needs
            # masks), then the big ones
            nc.gpsimd.dma_start(out=ident, in_=identD[:])
            nc.gpsimd.dma_start(out=masks, in_=masksD[:])
            nc.gpsimd.dma_start(out=v_aug[:, :, :, DH:2 * DH], in_=onesBlkD[:])
            nc.gpsimd.dma_start(
                out=wo, in_=woD[:].rearrange("p (db d) -> p db d", d=128))

            def dup_cossin_cols():
                for t in (cos2, sins2):
                    nc.gpsimd.dma_start(out=t[:, N:2 * N], in_=t[:, 0:N])

            def p1_chunk(ch):
                """QKV + RoPE + V transpose for one 512-token chunk."""
                t0 = ch * CHUNK
                if ch == 0:
                    xt = xt0
                else:
                    xt = sb.tile([128, NCB, CHUNK], BF16, tag="xt", bufs=8,
                                 name=f"xt{ch}")
                    half = NCB // 2
                    for hb in range(2):
                        nc.sync.dma_start(
                            out=xt[:, hb * half:(hb + 1) * half, :],
                            in_=xTD[hb * half * 128:(hb + 1) * half * 128,
                                    t0:t0 + CHUNK].rearrange(
                                "(cb p) n -> p cb n", p=128))
                csl = slice(t0, t0 + CHUNK)
                for which, W in (("q", wq), ("k", wk), ("v", wv)):
                    pp = ps.tile([128, 2, CHUNK], F32, tag="big", bufs=2,
                                 name=f"pp{ch}{which}")
                    for cb in range(NCB):
                        nc.tensor.matmul(pp[:, 0, :], W[:, cb, :],
                                         xt[:, cb, :],
                                         start=(cb == 0), stop=(cb == NCB - 1))
                    if which in ("q", "k"):
                        dst = qt if which == "q" else kt
                        raw = sb.tile([128, CHUNK], BF16, tag="raw", bufs=4,
                                      name=f"w{ch}{which}")
                        nc.vector.tensor_copy(raw, pp[:, 0, :])
                        rawsw = sb.tile([128, CHUNK], BF16, tag="rawsw", bufs=4,
                                        name=f"x{ch}{which}")
                        for hh in (0, 64):
                            a, bnd, c2 = hh, hh + 32, hh + 64
                            nc.gpsimd.dma_start(out=rawsw[a:bnd, :],
                                                in_=raw[bnd:c2, :])
                            nc.gpsimd.dma_start(out=rawsw[bnd:c2, :],
                                                in_=raw[a:bnd, :])
                        tmp = sb.tile([128, CHUNK], BF16, tag="tmp", bufs=3,
                                      name=f"t{ch}{which}")
                        tmp2 = sb.tile([128, CHUNK], BF16, tag="tmp2", bufs=3,
                                       name=f"u{ch}{which}")
                        nc.vector.tensor_mul(tmp, raw, cos2[:, csl])
                        nc.vector.tensor_mul(tmp2, rawsw, sins2[:, csl])
                        nc.vector.tensor_add(dst[:, csl], tmp, tmp2)
                    else:
                        vtc = sb.tile([128, CHUNK], BF16, tag="vtc", bufs=3,
                                      name=f"v{ch}")
                        nc.vector.tensor_copy(vtc, pp[:, 0, :])
                        bidx = ch // 4
                        for tb in range(4):
                            jb = (ch % 4) * 4 + tb
                            pt = ps.tile([128, 128], BF16, tag="sm",
                                         bufs=2, name=f"pt{ch}{tb}")
                            nc.tensor.transpose(
                                pt, vtc[:, tb * 128:(tb + 1) * 128], ident)
                            for h in range(HPC):
                                nc.vector.tensor_copy(
                                    v_aug[:, bidx * HPC + h, jb, 0:DH],
                                    pt[:, h * DH:(h + 1) * DH])

            ots = {}

            def p2_attn(bidx, ch):
                """Attention for i-chunk ch of batch bidx -> ot tile."""
                gcol = bidx * N + ch * CHUNK
                njb = 4 * (ch + 1)
                ot = sb.tile([128, CHUNK], BF16, tag="ot", bufs=3,
                             name=f"ot{bidx}{ch}")
                ots[(bidx, ch)] = ot
                for h in range(HPC):
                    inst = bidx * HPC + h
                    qr = slice(h * 64, (h + 1) * 64)
                    po = ps.tile([128, CHUNK], F32, tag="po", bufs=2,
                                 name=f"po{bidx}{ch}{h}")
                    for r0 in range(0, njb, 2):
                        pst = ps.tile([128, 2, CHUNK], F32, tag="big", bufs=2,
                                      name=f"ps{bidx}{ch}{h}{r0}")
                        # causal-tight: j-block jb only contributes to
                        # i-columns >= (jb - 4*ch)*128 within this chunk.
                        # S matmuls first, then both mask matmuls (adjacent
                        # mask matmuls share the ident stationary).
                        for idx in range(2):
                            jb = r0 + idx
                            jc = bidx * N + jb * 128
                            diag = jb >= 4 * ch
                            i0 = max(0, (jb - 4 * ch) * 128)
                            nc.tensor.matmul(
                                pst[:, idx, i0:CHUNK], kt[qr, jc:jc + 128],
                                qt[qr, gcol + i0:gcol + CHUNK],
                                start=True, stop=not diag)
                            if diag:
                                # triangular mask only on the 128-col stripe
                                nc.tensor.matmul(
                                    pst[:, idx, i0:i0 + 128], ident, masks,
                                    start=False, stop=True)
                        i0p = max(0, (r0 - 4 * ch) * 128)
                        expt = sb.tile([128, 2, CHUNK], BF16, tag="expt",
                                       bufs=6, name=f"e{bidx}{ch}{h}{r0}")
                        nc.scalar.activation(expt[:, :, i0p:CHUNK],
                                             pst[:, :, i0p:CHUNK], Exp)
                        for idx in range(2):
                            jb = r0 + idx
                            i0 = max(0, (jb - 4 * ch) * 128)
                            nc.tensor.matmul(
                                po[:, i0:CHUNK], v_aug[:, inst, jb, :],
                                expt[:, idx, i0:CHUNK],
                                start=(jb == 0), stop=(jb == njb - 1))
                    # normalize: sums are replicated in po rows 64:128.
                    # 1/s via integer-magic seed + one Newton pass (max rel
                    # err 3.4e-3) -- ~2.5x cheaper than DVE InstReciprocal
                    # and pipelines as independent short ops.
                    y0 = sb.tile([DH, CHUNK], F32, tag="y0", bufs=2,
                                 name=f"y{bidx}{ch}{h}")
                    nc.vector.tensor_scalar(
                        y0[:].bitcast(I32), po[DH:2 * DH, :].bitcast(I32),
                        0x7EF127EA, -1,
                        mybir.AluOpType.subtract, mybir.AluOpType.mult)
                    u = sb.tile([DH, CHUNK], F32, tag="u", bufs=2,
                                name=f"u{bidx}{ch}{h}")
                    nc.vector.tensor_mul(u, po[0:DH, :], y0)
                    t = sb.tile([DH, CHUNK], F32, tag="tn", bufs=2,
                                name=f"t{bidx}{ch}{h}")
                    nc.vector.tensor_mul(t, po[DH:2 * DH, :], y0)
                    t2 = sb.tile([DH, CHUNK], F32, tag="t2", bufs=2,
                                 name=f"s{bidx}{ch}{h}")
                    nc.vector.tensor_scalar(
                        t2, t, -1.0, 2.0,
                        mybir.AluOpType.mult, mybir.AluOpType.add)
                    nc.vector.tensor_mul(ot[qr, :], u, t2)
            def p2_wo(bidx, ch, split=False):
                """Output projection for i-chunk ch of batch bidx."""
                gcol = bidx * N + ch * CHUNK
                ot = ots.pop((bidx, ch))
                for db in range(NCB):
                    ppr = ps.tile([128, CHUNK], F32, tag="sm", bufs=2,
                                  name=f"pj{bidx}{ch}{db}")
                    if split:
                        # per-head accumulation: h0's Wo runs while h1's
                        # normalize is in flight (cuts the kernel tail)
                        for hh in range(HPC):
                            hr = slice(hh * 64, (hh + 1) * 64)
                            nc.tensor.matmul(ppr, wo[hr, db, :], ot[hr, :],
                                             start=(hh == 0), stop=(hh == 1))
                    else:
                        nc.tensor.matmul(ppr, wo[:, db, :], ot,
                                         start=True, stop=True)
                    osb = sb.tile([128, CHUNK], BF16, tag="osb", bufs=4,
                                  name=f"o{bidx}{ch}{db}")
                    if db % 2 == 0:
                        nc.vector.tensor_copy(osb, ppr)
                    else:
                        nc.scalar.activation(
                            osb, ppr, mybir.ActivationFunctionType.Copy)
                    nc.sync.dma_start(
                        out=outD[db * 128:(db + 1) * 128, gcol:gcol + CHUNK],
                        in_=osb)

            for _ in range(reps):
                p1_chunk(0)
                p1_chunk(1)
                p2_attn(0, 0)
                p2_wo(0, 0)
                p1_chunk(2)
                dup_cossin_cols()
                p2_attn(0, 1)
                p2_wo(0, 1)
                p1_chunk(3)
                p2_attn(0, 2)
                p2_wo(0, 2)
                p1_chunk(4)
                p2_attn(0, 3)
                p2_wo(0, 3)
                p1_chunk(5)
                p2_attn(1, 0)
                p2_wo(1, 0)
                p1_chunk(6)
                p2_attn(1, 1)
                p2_wo(1, 1)
                p1_chunk(7)
                p2_attn(1, 2)
                p2_wo(1, 2)
                p2_attn(1, 3)
                p2_wo(1, 3)
    nc.compile()
    return nc


def _get_nc(reps=1):
    if reps not in _NC_CACHE:
        _NC_CACHE[reps] = build(reps)
    return _NC_CACHE[reps]


def make_in_maps(x, pos_emb, Wq, Wk, Wv, Wo):
    x = np.asarray(x, np.float32)
    pos_emb = np.asarray(pos_emb, np.float32)
    Wq = np.asarray(Wq, np.float32)
    Wk = np.asarray(Wk, np.float32)
    Wv = np.asarray(Wv, np.float32)
    Wo = np.asarray(Wo, np.float32)

    xT = np.ascontiguousarray(x.reshape(T, DIM).T).astype(NPBF16)  # [DIM, T]
    cosT1 = np.cos(pos_emb).T                            # [DH, N]
    cosT = np.ascontiguousarray(np.tile(cosT1, (2, 1))).astype(NPBF16)
    sinT = np.sin(pos_emb).T
    sins1 = np.concatenate([-sinT[0:32], sinT[32:64]], axis=0)
    sinsT = np.ascontiguousarray(np.tile(sins1, (2, 1))).astype(NPBF16)

    ident = np.eye(128, dtype=NPBF16)
    scale = np.float32(DH ** -0.5)
    jj = np.arange(128)[:, None]
    ii = np.arange(128)[None, :]
    masks = np.where(jj <= ii, 0.0, NEG).astype(NPBF16)
    ones_blk = np.ones((128, NINST, NJB, DH), NPBF16)

    in_maps = []
    for c in range(NCORES):
        cols = slice(c * 128, (c + 1) * 128)
        in_maps.append(dict(
            xT=xT,
            wq=(np.ascontiguousarray(Wq[:, cols]) * scale).astype(NPBF16),
            wk=np.ascontiguousarray(Wk[:, cols]).astype(NPBF16),
            wv=np.ascontiguousarray(Wv[:, cols]).astype(NPBF16),
            wo=np.ascontiguousarray(Wo[cols, :]).astype(NPBF16),
            cosT=cosT, sinsT=sinsT, identD=ident,
            masksD=masks, onesBlkD=ones_blk,
        ))
    return in_maps


def run(in_maps, trace=False, reps=1, **kw):
    nc = _get_nc(reps)
    return run_bass_kernel_spmd(nc, in_maps, list(range(NCORES)),
                                trace=trace, **kw)


def kernel(x, pos_emb, Wq, Wk, Wv, Wo, bo):
    in_maps = make_in_maps(x, pos_emb, Wq, Wk, Wv, Wo)
    res = run(in_maps)
    acc = np.zeros((DIM, T), np.float64)
    for c in range(NCORES):
        acc += np.asarray(res.results[c]["outT"], np.float32)
    out = acc.T.reshape(B, N, DIM) + np.asarray(bo, np.float32)[None, None, :]
    return out.astype(np.float32)


# revision 46
# speedup vs baseline: 1.1823x; 1.0145x over previous
"""Causal multi-head attention with RoPE on 8 Trainium2 NeuronCores.

Problem: B=2, N=2048, DIM=1024, H=16, DH=64, fp32 in/out.
Sharding: head-parallel — core c owns heads 2c, 2c+1 (columns c*128:(c+1)*128
of Wq/Wk/Wv, rows c*128:(c+1)*128 of Wo) for both batches. Each core computes
its partial output projection [DIM, B*N]; the host sums the 8 partials
(the "all-reduce") and adds the bias.

All matmul operands are bf16 (fp32r at full PE rate draws enough power to
trigger sustained DVS throttling — util capped near 50% — so bf16 is ~2x
faster in practice and halves LDWEIGHTS/SBUF/DMA traffic). PSUM accumulation
stays fp32; tolerance is 2e-2 so bf16 operands are comfortably accurate.

On-device pipeline:
  p1(chunk): QKV projections (K=1024 accum in PSUM), RoPE applied from PSUM
             via DVE in bf16 -> qT/kT resident [128, 4096]; V transposed via
             PE ([128,128] per token block, both heads at once) into [j, d]
             layout, augmented with a 64-col ones BLOCK.
  p2(b, ch): per head: S^T = k q^T per 128-j-block, causal-tight (columns
             below the diagonal are never computed; the in-block triangle is
             masked in-PSUM via an identity-x-mask matmul on the 128-col
             diagonal stripe only); exp on ACT -> bf16; augmented V-matmul
             accumulates O'^T (rows 0:64) and the softmax sum replicated
             across rows 64:128 (ones block); normalize computes 1/s with an
             integer-magic seed + one Newton step on DVE (~2.5x cheaper than
             InstReciprocal); fused Wo projection per token chunk, with the
             PSUM->SBUF output copies split between DVE and ACT.
Emission interleaves p1 chunks with p2 so DVE-heavy RoPE overlaps PE-heavy
attention, keeping ~1.5 chunks of lookahead (tighter coupling measurably
stalls the PE on RoPE latency). All x chunks are prefetched (bufs=8); input
DMAs are spread across the sync/scalar/gpsimd queues so x loads are never
stuck behind constants. Single PSUM pool (8 banks): pst 4, po 2, sm 2.
"""
import numpy as np
import ml_dtypes
import bass_rust
from concourse import bacc
import concourse.mybir as mybir
from concourse.tile import TileContext
from concourse.bass_utils import run_bass_kernel_spmd

B, N, DIM, H, DH = 2, 2048, 1024, 16, 64
NCORES = 8
HPC = H // NCORES          # 2 heads per core
T = B * N                  # 4096 tokens
CHUNK = 512
NCH = T // CHUNK           # 8 token chunks
NCB = DIM // 128           # 8 contraction blocks
NINST = B * HPC            # 4 attention instances per core
NJB = N // 128             # 16 j-blocks per batch
NEG = -1e9

F32 = mybir.dt.float32
I32 = mybir.dt.int32
BF16 = mybir.dt.bfloat16
NPBF16 = ml_dtypes.bfloat16

_NC_CACHE = {}


def build(reps=1):
    nc = bacc.Bacc()
    xTD = nc.dram_tensor("xT", [DIM, T], BF16, kind="ExternalInput")
    wqD = nc.dram_tensor("wq", [DIM, 128], BF16, kind="ExternalInput")
    wkD = nc.dram_tensor("wk", [DIM, 128], BF16, kind="ExternalInput")
    wvD = nc.dram_tensor("wv", [DIM, 128], BF16, kind="ExternalInput")
    woD = nc.dram_tensor("wo", [128, DIM], BF16, kind="ExternalInput")
    cosD = nc.dram_tensor("cosT", [2 * DH, N], BF16, kind="ExternalInput")
    sinsD = nc.dram_tensor("sinsT", [2 * DH, N], BF16, kind="ExternalInput")
    identD = nc.dram_tensor("identD", [128, 128], BF16, kind="ExternalInput")
    masksD = nc.dram_tensor("masksD", [128, 128], BF16, kind="ExternalInput")
    onesBlkD = nc.dram_tensor("onesBlkD", [128, NINST, NJB, DH], BF16,
                              kind="ExternalInput")
    outD = nc.dram_tensor("outT", [DIM, T], BF16, kind="ExternalOutput")

    Exp = mybir.ActivationFunctionType.Exp

    with TileContext(nc) as tc:
        with (
            tc.tile_pool(name="const", bufs=1) as cp,
            tc.tile_pool(name="sb", bufs=2) as sb,
            tc.tile_pool(name="ps", bufs=1, space="PSUM") as ps,
        ):
            ident = cp.tile([128, 128], BF16)
            masks = cp.tile([128, 128], BF16)
            wq = cp.tile([128, NCB, 128], BF16)
            wk = cp.tile([128, NCB, 128], BF16)
            wv = cp.tile([128, NCB, 128], BF16)
            wo = cp.tile([128, NCB, 128], BF16)
            cos2 = cp.tile([128, T], BF16)
            sins2 = cp.tile([128, T], BF16)
            qt = cp.tile([128, T], BF16)     # rows 0:64 head0, 64:128 head1
            kt = cp.tile([128, T], BF16)
            # per (inst, jb): cols 0:64 = V^T block, cols 64:128 = ones
            v_aug = cp.tile([128, NINST, NJB, 2 * DH], BF16)

            # minimal front-load: wq, then chunk-0 x, then the rest
            nc.sync.dma_start(
                out=wq[:, 0:1, :],
                in_=wqD[0:128, :].rearrange("(cb p) d -> p cb d", p=128))
            nc.sync.dma_start(
                out=wq[:, 1:NCB, :],
                in_=wqD[128:DIM, :].rearrange("(cb p) d -> p cb d", p=128))
            xt0 = sb.tile([128, NCB, CHUNK], BF16, tag="xt", bufs=8,
                          name="xt_first")
            nc.sync.dma_start(out=xt0[:, 0:1, :],
                              in_=xTD[0:128, 0:CHUNK].rearrange(
                                  "(cb p) n -> p cb n", p=128))
            nc.sync.dma_start(out=xt0[:, 1:4, :],
                              in_=xTD[128:512, 0:CHUNK].rearrange(
                                  "(cb p) n -> p cb n", p=128))
            nc.sync.dma_start(out=xt0[:, 4:8, :],
                              in_=xTD[512:1024, 0:CHUNK].rearrange(
                                  "(cb p) n -> p cb n", p=128))
            # chunk-0 slice of cos/sin first (tiny, unblocks RoPE), then
            # weights, then the rest of cos/sin
            nc.scalar.dma_start(out=cos2[:, 0:CHUNK], in_=cosD[:, 0:CHUNK])
            nc.scalar.dma_start(out=sins2[:, 0:CHUNK], in_=sinsD[:, 0:CHUNK])
            for t, d in ((wk, wkD), (wv, wvD)):
                nc.scalar.dma_start(
                    out=t, in_=d[:].rearrange("(cb p) d -> p cb d", p=128))
            nc.scalar.dma_start(out=cos2[:, CHUNK:N], in_=cosD[:, CHUNK:N])
            nc.scalar.dma_start(out=sins2[:, CHUNK:N], in_=sinsD[:, CHUNK:N])

            # deferred constants (p2/transposes only) ride the SWDGE queues;
            # small tiles first (chunk-0 transposes need ident, first S needs
            # masks), then the big ones
            nc.gpsimd.dma_start(out=ident, in_=identD[:])
            nc.gpsimd.dma_start(out=masks, in_=masksD[:])
            nc.gpsimd.dma_start(out=v_aug[:, :, :, DH:2 * DH], in_=onesBlkD[:])
            nc.gpsimd.dma_start(
                out=wo, in_=woD[:].rearrange("p (db d) -> p db d", d=128))

            def dup_cossin_cols():
                for t in (cos2, sins2):
                    nc.gpsimd.dma_start(out=t[:, N:2 * N], in_=t[:, 0:N])

            def p1_chunk(ch):
                """QKV + RoPE + V transpose for one 512-token chunk."""
                t0 = ch * CHUNK
                if ch == 0:
                    xt = xt0
                else:
                    xt = sb.tile([128, NCB, CHUNK], BF16, tag="xt", bufs=8,
                                 name=f"xt{ch}")
                    half = NCB // 2
                    for hb in range(2):
                        nc.sync.dma_start(
                            out=xt[:, hb * half:(hb + 1) * half, :],
                            in_=xTD[hb * half * 128:(hb + 1) * half * 128,
                                    t0:t0 + CHUNK].rearrange(
                                "(cb p) n -> p cb n", p=128))
                csl = slice(t0, t0 + CHUNK)
                for which, W in (("q", wq), ("k", wk), ("v", wv)):
                    pp = ps.tile([128, 2, CHUNK], F32, tag="big", bufs=2,
                                 name=f"pp{ch}{which}")
                    for cb in range(NCB):
                        nc.tensor.matmul(pp[:, 0, :], W[:, cb, :],
                                         xt[:, cb, :],
                                         start=(cb == 0), stop=(cb == NCB - 1))
                    if which in ("q", "k"):
                        dst = qt if which == "q" else kt
                        raw = sb.tile([128, CHUNK], BF16, tag="raw", bufs=4,
                                      name=f"w{ch}{which}")
                        nc.vector.tensor_copy(raw, pp[:, 0, :])
                        rawsw = sb.tile([128, CHUNK], BF16, tag="rawsw", bufs=4,
                                        name=f"x{ch}{which}")
                        for hh in (0, 64):
                            a, bnd, c2 = hh, hh + 32, hh + 64
                            nc.gpsimd.dma_start(out=rawsw[a:bnd, :],
                                                in_=raw[bnd:c2, :])
                            nc.gpsimd.dma_start(out=rawsw[bnd:c2, :],
                                                in_=raw[a:bnd, :])
                        tmp = sb.tile([128, CHUNK], BF16, tag="tmp", bufs=3,
                                      name=f"t{ch}{which}")
                        tmp2 = sb.tile([128, CHUNK], BF16, tag="tmp2", bufs=3,
                                       name=f"u{ch}{which}")
                        nc.vector.tensor_mul(tmp, raw, cos2[:, csl])
                        nc.vector.tensor_mul(tmp2, rawsw, sins2[:, csl])
                        nc.vector.tensor_add(dst[:, csl], tmp, tmp2)
                    else:
                        vtc = sb.tile([128, CHUNK], BF16, tag="vtc", bufs=3,
                                      name=f"v{ch}")
                        nc.vector.tensor_copy(vtc, pp[:, 0, :])
                        bidx = ch // 4
                        for tb in range(4):
                            jb = (ch % 4) * 4 + tb
                            pt = ps.tile([128, 128], BF16, tag="sm",
                                         bufs=2, name=f"pt{ch}{tb}")
                            nc.tensor.transpose(
                                pt, vtc[:, tb * 128:(tb + 1) * 128], ident)
                            for h in range(HPC):
                                nc.vector.tensor_copy(
                                    v_aug[:, bidx * HPC + h, jb, 0:DH],
                                    pt[:, h * DH:(h + 1) * DH])

            ots = {}

            def p2_attn(bidx, ch):
                """Attention for i-chunk ch of batch bidx -> ot tile."""
                gcol = bidx * N + ch * CHUNK
                njb = 4 * (ch + 1)
                ot = sb.tile([128, CHUNK], BF16, tag="ot", bufs=3,
                             name=f"ot{bidx}{ch}")
                ots[(bidx, ch)] = ot
                for h in range(HPC):
                    inst = bidx * HPC + h
                    qr = slice(h * 64, (h + 1) * 64)
                    po = ps.tile([128, CHUNK], F32, tag="po", bufs=2,
                                 name=f"po{bidx}{ch}{h}")
                    for r0 in range(0, njb, 2):
                        pst = ps.tile([128, 2, CHUNK], F32, tag="big", bufs=2,
                                      name=f"ps{bidx}{ch}{h}{r0}")
                        # causal-tight: j-block jb only contributes to
                        # i-columns >= (jb - 4*ch)*128 within this chunk.
                        # S matmuls first, then both mask matmuls (adjacent
                        # mask matmuls share the ident stationary).
                        for idx in range(2):
                            jb = r0 + idx
                            jc = bidx * N + jb * 128
                            diag = jb >= 4 * ch
                            i0 = max(0, (jb - 4 * ch) * 128)
                            nc.tensor.matmul(
                                pst[:, idx, i0:CHUNK], kt[qr, jc:jc + 128],
                                qt[qr, gcol + i0:gcol + CHUNK],
                                start=True, stop=not diag)
                            if diag:
                                # triangular mask only on the 128-col stripe
                                nc.tensor.matmul(
                                    pst[:, idx, i0:i0 + 128], ident, masks,
                                    start=False, stop=True)
                        i0p = max(0, (r0 - 4 * ch) * 128)
                        expt = sb.tile([128, 2, CHUNK], BF16, tag="expt",
                                       bufs=6, name=f"e{bidx}{ch}{h}{r0}")
                        nc.scalar.activation(expt[:, :, i0p:CHUNK],
                                             pst[:, :, i0p:CHUNK], Exp)
                        for idx in range(2):
                            jb = r0 + idx
                            i0 = max(0, (jb - 4 * ch) * 128)
                            nc.tensor.matmul(
                                po[:, i0:CHUNK], v_aug[:, inst, jb, :],
                                expt[:, idx, i0:CHUNK],
                                start=(jb == 0), stop=(jb == njb - 1))
                    # normalize: sums are replicated in po rows 64:128.
                    # 1/s via integer-magic seed + one Newton pass (max rel
                    # err 3.4e-3) -- ~2.5x cheaper than DVE InstReciprocal
                    # and pipelines as independent short ops.
                    y0 = sb.tile([DH, CHUNK], F32, tag="y0", bufs=2,
                                 name=f"y{bidx}{ch}{h}")
                    nc.vector.tensor_scalar(
                        y0[:].bitcast(I32), po[DH:2 * DH, :].bitcast(I32),
                        0x7EF127EA, -1,
                        mybir.AluOpType.subtract, mybir.AluOpType.mult)
                    u = sb.tile([DH, CHUNK], F32, tag="u", bufs=2,
                                name=f"u{bidx}{ch}{h}")
                    nc.vector.tensor_mul(u, po[0:DH, :], y0)
                    t = sb.tile([DH, CHUNK], F32, tag="tn", bufs=2,
                                name=f"t{bidx}{ch}{h}")
                    nc.vector.tensor_mul(t, po[DH:2 * DH, :], y0)
                    t2 = sb.tile([DH, CHUNK], F32, tag="t2", bufs=2,
                                 name=f"s{bidx}{ch}{h}")
                    nc.vector.tensor_scalar(
                        t2, t, -1.0, 2.0,
                        mybir.AluOpType.mult, mybir.AluOpType.add)
                    nc.vector.tensor_mul(ot[qr, :], u, t2)
            def p2_wo(bidx, ch, split=False):
                """Output projection for i-chunk ch of batch bidx."""
                gcol = bidx * N + ch * CHUNK
                ot = ots.pop((bidx, ch))
                for db in range(NCB):
                    ppr = ps.tile([128, CHUNK], F32, tag="sm", bufs=2,
                                  name=f"pj{bidx}{ch}{db}")
                    if split:
                        # per-head accumulation: h0's Wo runs while h1's
                        # normalize is in flight (cuts the kernel tail)
                        for hh in range(HPC):
                            hr = slice(hh * 64, (hh + 1) * 64)
                            nc.tensor.matmul(ppr, wo[hr, db, :], ot[hr, :],
                                             start=(hh == 0), stop=(hh == 1))
                    else:
                        nc.tensor.matmul(ppr, wo[:, db, :], ot,
                                         start=True, stop=True)
                    osb = sb.tile([128, CHUNK], BF16, tag="osb", bufs=4,
                                  name=f"o{bidx}{ch}{db}")
                    if db % 2 == 0:
                        nc.vector.tensor_copy(osb, ppr)
                    else:
                        nc.scalar.activation(
                            osb, ppr, mybir.ActivationFunctionType.Copy)
                    nc.sync.dma_start(
                        out=outD[db * 128:(db + 1) * 128, gcol:gcol + CHUNK],
                        in_=osb)

            for _ in range(reps):
                p1_chunk(0)
                p1_chunk(1)
                p2_attn(0, 0)
                p2_wo(0, 0)
                p1_chunk(2)
                dup_cossin_cols()
                p2_attn(0, 1)
                p2_wo(0, 1)
                p1_chunk(3)
                p2_attn(0, 2)
                p2_wo(0, 2)
                p1_chunk(4)
                p2_attn(0, 3)
                p2_wo(0, 3)
                p1_chunk(5)
                p2_attn(1, 0)
                p2_wo(1, 0)
                p1_chunk(6)
                p2_attn(1, 1)
                p2_wo(1, 1)
                p1_chunk(7)
                p2_attn(1, 2)
                p2_attn(1, 3)
                p2_wo(1, 2)
                p2_wo(1, 3)
    nc.compile()
    return nc


def _get_nc(reps=1):
    if reps not in _NC_CACHE:
        _NC_CACHE[reps] = build(reps)
    return _NC_CACHE[reps]


def make_in_maps(x, pos_emb, Wq, Wk, Wv, Wo):
    x = np.asarray(x, np.float32)
    pos_emb = np.asarray(pos_emb, np.float32)
    Wq = np.asarray(Wq, np.float32)
    Wk = np.asarray(Wk, np.float32)
    Wv = np.asarray(Wv, np.float32)
    Wo = np.asarray(Wo, np.float32)

    xT = np.ascontiguousarray(x.reshape(T, DIM).T).astype(NPBF16)  # [DIM, T]
    cosT1 = np.cos(pos_emb).T                            # [DH, N]
    cosT = np.ascontiguousarray(np.tile(cosT1, (2, 1))).astype(NPBF16)
    sinT = np.sin(pos_emb).T
    sins1 = np.concatenate([-sinT[0:32], sinT[32:64]], axis=0)
    sinsT = np.ascontiguousarray(np.tile(sins1, (2, 1))).astype(NPBF16)

    ident = np.eye(128, dtype=NPBF16)
    scale = np.float32(DH ** -0.5)
    jj = np.arange(128)[:, None]
    ii = np.arange(128)[None, :]
    masks = np.where(jj <= ii, 0.0, NEG).astype(NPBF16)
    ones_blk = np.ones((128, NINST, NJB, DH), NPBF16)

    in_maps = []
    for c in range(NCORES):
        cols = slice(c * 128, (c + 1) * 128)
        in_maps.append(dict(
            xT=xT,
            wq=(np.ascontiguousarray(Wq[:, cols]) * scale).astype(NPBF16),
            wk=np.ascontiguousarray(Wk[:, cols]).astype(NPBF16),
            wv=np.ascontiguousarray(Wv[:, cols]).astype(NPBF16),
            wo=np.ascontiguousarray(Wo[cols, :]).astype(NPBF16),
            cosT=cosT, sinsT=sinsT, identD=ident,
            masksD=masks, onesBlkD=ones_blk,
        ))
    return in_maps


def run(in_maps, trace=False, reps=1, **kw):
    nc = _get_nc(reps)
    return run_bass_kernel_spmd(nc, in_maps, list(range(NCORES)),
                                trace=trace, **kw)


def kernel(x, pos_emb, Wq, Wk, Wv, Wo, bo):
    in_maps = make_in_maps(x, pos_emb, Wq, Wk, Wv, Wo)
    res = run(in_maps)
    acc = np.zeros((DIM, T), np.float64)
    for c in range(NCORES):
        acc += np.asarray(res.results[c]["outT"], np.float32)
    out = acc.T.reshape(B, N, DIM) + np.asarray(bo, np.float32)[None, None, :]
    return out.astype(np.float32)


# revision 47
# speedup vs baseline: 1.3705x; 1.1592x over previous
"""Causal multi-head attention with RoPE on 8 Trainium2 NeuronCores.

Problem: B=2, N=2048, DIM=1024, H=16, DH=64, fp32 in/out.
Sharding: head-parallel — core c owns heads 2c, 2c+1 (columns c*128:(c+1)*128
of Wq/Wk/Wv, rows c*128:(c+1)*128 of Wo) for both batches. Each core computes
its partial output projection [DIM, B*N]; the host sums the 8 partials
(the "all-reduce") and adds the bias.

All matmul operands are bf16 (fp32r at full PE rate draws enough power to
trigger sustained DVS throttling — util capped near 50% — so bf16 is ~2x
faster in practice and halves LDWEIGHTS/SBUF/DMA traffic). PSUM accumulation
stays fp32; tolerance is 2e-2 so bf16 operands are comfortably accurate.

On-device pipeline:
  p1(chunk): QKV projections (K=1024 accum in PSUM), RoPE applied from PSUM
             via DVE in bf16 -> qT/kT resident [128, 4096]; V transposed via
             PE ([128,128] per token block, both heads at once) into [j, d]
             layout, augmented with a 64-col ones BLOCK.
  p2(b, ch): per head: S^T = k q^T per 128-j-block, causal-tight (columns
             below the diagonal are never computed; the in-block triangle is
             masked in-PSUM via an identity-x-mask matmul on the 128-col
             diagonal stripe only); exp on ACT -> bf16; augmented V-matmul
             accumulates O'^T (rows 0:64) and the softmax sum replicated
             across rows 64:128 (ones block); normalize computes 1/s with an
             integer-magic seed + one Newton step on DVE (~2.5x cheaper than
             InstReciprocal); fused Wo projection per token chunk, with the
             PSUM->SBUF output copies split between DVE and ACT.
Emission interleaves p1 chunks with p2 so DVE-heavy RoPE overlaps PE-heavy
attention, keeping ~1.5 chunks of lookahead (tighter coupling measurably
stalls the PE on RoPE latency). All x chunks are prefetched (bufs=8); input
DMAs are spread across the sync/scalar/gpsimd queues so x loads are never
stuck behind constants. Single PSUM pool (8 banks): pst 4, po 2, sm 2.
"""
import numpy as np
import ml_dtypes
import bass_rust
from concourse import bacc
import concourse.mybir as mybir
from concourse.tile import TileContext
from concourse.bass_utils import run_bass_kernel_spmd

B, N, DIM, H, DH = 2, 2048, 1024, 16, 64
NCORES = 8
HPC = H // NCORES          # 2 heads per core
T = B * N                  # 4096 tokens
CHUNK = 512
NCH = T // CHUNK           # 8 token chunks
NCB = DIM // 128           # 8 contraction blocks
NINST = B * HPC            # 4 attention instances per core
NJB = N // 128             # 16 j-blocks per batch
NEG = -1e9

F32 = mybir.dt.float32
I32 = mybir.dt.int32
BF16 = mybir.dt.bfloat16
NPBF16 = ml_dtypes.bfloat16

_NC_CACHE = {}


def build(reps=1):
    nc = bacc.Bacc()
    xTD = nc.dram_tensor("xT", [DIM, T], BF16, kind="ExternalInput")
    wqD = nc.dram_tensor("wq", [DIM, 128], BF16, kind="ExternalInput")
    wkD = nc.dram_tensor("wk", [DIM, 128], BF16, kind="ExternalInput")
    wvD = nc.dram_tensor("wv", [DIM, 128], BF16, kind="ExternalInput")
    woD = nc.dram_tensor("wo", [128, DIM], BF16, kind="ExternalInput")
    cosD = nc.dram_tensor("cosT", [2 * DH, N], BF16, kind="ExternalInput")
    sinsD = nc.dram_tensor("sinsT", [2 * DH, N], BF16, kind="ExternalInput")
    identD = nc.dram_tensor("identD", [128, 128], BF16, kind="ExternalInput")
    masksD = nc.dram_tensor("masksD", [128, 128], BF16, kind="ExternalInput")
    onesBlkD = nc.dram_tensor("onesBlkD", [128, NINST, NJB, DH], BF16,
                              kind="ExternalInput")
    outD = nc.dram_tensor("outT", [DIM, T], BF16, kind="ExternalOutput")

    Exp = mybir.ActivationFunctionType.Exp

    with TileContext(nc) as tc:
        with (
            tc.tile_pool(name="const", bufs=1) as cp,
            tc.tile_pool(name="sb", bufs=2) as sb,
            tc.tile_pool(name="ps", bufs=1, space="PSUM") as ps,
        ):
            ident = cp.tile([128, 128], BF16)
            masks = cp.tile([128, 128], BF16)
            wq = cp.tile([128, NCB, 128], BF16)
            wk = cp.tile([128, NCB, 128], BF16)
            wv = cp.tile([128, NCB, 128], BF16)
            wo = cp.tile([128, NCB, 128], BF16)
            cos2 = cp.tile([128, T], BF16)
            sins2 = cp.tile([128, T], BF16)
            qt = cp.tile([128, T], BF16)     # rows 0:64 head0, 64:128 head1
            kt = cp.tile([128, T], BF16)
            # per (inst, jb): cols 0:64 = V^T block, cols 64:128 = ones
            v_aug = cp.tile([128, NINST, NJB, 2 * DH], BF16)

            # minimal front-load: wq, then chunk-0 x, then the rest
            nc.sync.dma_start(
                out=wq[:, 0:1, :],
                in_=wqD[0:128, :].rearrange("(cb p) d -> p cb d", p=128))
            nc.sync.dma_start(
                out=wq[:, 1:NCB, :],
                in_=wqD[128:DIM, :].rearrange("(cb p) d -> p cb d", p=128))
            xt0 = sb.tile([128, NCB, CHUNK], BF16, tag="xt", bufs=8,
                          name="xt_first")
            nc.sync.dma_start(out=xt0[:, 0:1, :],
                              in_=xTD[0:128, 0:CHUNK].rearrange(
                                  "(cb p) n -> p cb n", p=128))
            nc.sync.dma_start(out=xt0[:, 1:4, :],
                              in_=xTD[128:512, 0:CHUNK].rearrange(
                                  "(cb p) n -> p cb n", p=128))
            nc.sync.dma_start(out=xt0[:, 4:8, :],
                              in_=xTD[512:1024, 0:CHUNK].rearrange(
                                  "(cb p) n -> p cb n", p=128))
            # chunk-0 slice of cos/sin first (tiny, unblocks RoPE), then
            # weights, then the rest of cos/sin
            nc.scalar.dma_start(out=cos2[:, 0:CHUNK], in_=cosD[:, 0:CHUNK])
            nc.scalar.dma_start(out=sins2[:, 0:CHUNK], in_=sinsD[:, 0:CHUNK])
            for t, d in ((wk, wkD), (wv, wvD)):
                nc.scalar.dma_start(
                    out=t, in_=d[:].rearrange("(cb p) d -> p cb d", p=128))
            nc.scalar.dma_start(out=cos2[:, CHUNK:N], in_=cosD[:, CHUNK:N])
            nc.scalar.dma_start(out=sins2[:, CHUNK:N], in_=sinsD[:, CHUNK:N])

            # deferred constants (p2/transposes only) ride the SWDGE queues;
            # small tiles first (chunk-0 transposes need ident, first S needs
            # masks), then the big ones
            nc.gpsimd.dma_start(out=ident, in_=identD[:])
            nc.gpsimd.dma_start(out=masks, in_=masksD[:])
            nc.gpsimd.dma_start(out=v_aug[:, :, :, DH:2 * DH], in_=onesBlkD[:])
            nc.gpsimd.dma_start(
                out=wo, in_=woD[:].rearrange("p (db d) -> p db d", d=128))

            def dup_cossin_cols():
                for t in (cos2, sins2):
                    nc.gpsimd.dma_start(out=t[:, N:2 * N], in_=t[:, 0:N])

            def p1_chunk(ch):
                """QKV + RoPE + V transpose for one 512-token chunk."""
                t0 = ch * CHUNK
                if ch == 0:
                    xt = xt0
                else:
                    xt = sb.tile([128, NCB, CHUNK], BF16, tag="xt", bufs=8,
                                 name=f"xt{ch}")
                    half = NCB // 2
                    for hb in range(2):
                        nc.sync.dma_start(
                            out=xt[:, hb * half:(hb + 1) * half, :],
                            in_=xTD[hb * half * 128:(hb + 1) * half * 128,
                                    t0:t0 + CHUNK].rearrange(
                                "(cb p) n -> p cb n", p=128))
                csl = slice(t0, t0 + CHUNK)
                for which, W in (("q", wq), ("k", wk), ("v", wv)):
                    pp = ps.tile([128, 2, CHUNK], F32, tag="big", bufs=2,
                                 name=f"pp{ch}{which}")
                    for cb in range(NCB):
                        nc.tensor.matmul(pp[:, 0, :], W[:, cb, :],
                                         xt[:, cb, :],
                                         start=(cb == 0), stop=(cb == NCB - 1))
                    if which in ("q", "k"):
                        dst = qt if which == "q" else kt
                        raw = sb.tile([128, CHUNK], BF16, tag="raw", bufs=4,
                                      name=f"w{ch}{which}")
                        nc.vector.tensor_copy(raw, pp[:, 0, :])
                        rawsw = sb.tile([128, CHUNK], BF16, tag="rawsw", bufs=4,
                                        name=f"x{ch}{which}")
                        for hh in (0, 64):
                            a, bnd, c2 = hh, hh + 32, hh + 64
                            nc.gpsimd.dma_start(out=rawsw[a:bnd, :],
                                                in_=raw[bnd:c2, :])
                            nc.gpsimd.dma_start(out=rawsw[bnd:c2, :],
                                                in_=raw[a:bnd, :])
                        tmp = sb.tile([128, CHUNK], BF16, tag="tmp", bufs=3,
                                      name=f"t{ch}{which}")
                        tmp2 = sb.tile([128, CHUNK], BF16, tag="tmp2", bufs=3,
                                       name=f"u{ch}{which}")
                        nc.vector.tensor_mul(tmp, raw, cos2[:, csl])
                        nc.vector.tensor_mul(tmp2, rawsw, sins2[:, csl])
                        nc.vector.tensor_add(dst[:, csl], tmp, tmp2)
                    else:
                        vtc = sb.tile([128, CHUNK], BF16, tag="vtc", bufs=3,
                                      name=f"v{ch}")
                        nc.vector.tensor_copy(vtc, pp[:, 0, :])
                        bidx = ch // 4
                        for tb in range(4):
                            jb = (ch % 4) * 4 + tb
                            pt = ps.tile([128, 128], BF16, tag="sm",
                                         bufs=2, name=f"pt{ch}{tb}")
                            nc.tensor.transpose(
                                pt, vtc[:, tb * 128:(tb + 1) * 128], ident)
                            for h in range(HPC):
                                nc.vector.tensor_copy(
                                    v_aug[:, bidx * HPC + h, jb, 0:DH],
                                    pt[:, h * DH:(h + 1) * DH])

            ots = {}

            def p2_attn(bidx, ch):
                """Attention for i-chunk ch of batch bidx -> ot tile."""
                gcol = bidx * N + ch * CHUNK
                njb = 4 * (ch + 1)
                ot = sb.tile([128, CHUNK], BF16, tag="ot", bufs=3,
                             name=f"ot{bidx}{ch}")
                ots[(bidx, ch)] = ot
                for h in range(HPC):
                    inst = bidx * HPC + h
                    qr = slice(h * 64, (h + 1) * 64)
                    po = ps.tile([128, CHUNK], F32, tag="po", bufs=2,
                                 name=f"po{bidx}{ch}{h}")
                    for r0 in range(0, njb, 2):
                        pst = ps.tile([128, 2, CHUNK], F32, tag="big", bufs=2,
                                      name=f"ps{bidx}{ch}{h}{r0}")
                        # causal-tight: j-block jb only contributes to
                        # i-columns >= (jb - 4*ch)*128 within this chunk.
                        # S matmuls first, then both mask matmuls (adjacent
                        # mask matmuls share the ident stationary).
                        for idx in range(2):
                            jb = r0 + idx
                            jc = bidx * N + jb * 128
                            diag = jb >= 4 * ch
                            i0 = max(0, (jb - 4 * ch) * 128)
                            nc.tensor.matmul(
                                pst[:, idx, i0:CHUNK], kt[qr, jc:jc + 128],
                                qt[qr, gcol + i0:gcol + CHUNK],
                                start=True, stop=not diag)
                            if diag:
                                # triangular mask only on the 128-col stripe
                                nc.tensor.matmul(
                                    pst[:, idx, i0:i0 + 128], ident, masks,
                                    start=False, stop=True)
                        i0p = max(0, (r0 - 4 * ch) * 128)
                        expt = sb.tile([128, 2, CHUNK], BF16, tag="expt",
                                       bufs=6, name=f"e{bidx}{ch}{h}{r0}")
                        nc.scalar.activation(expt[:, :, i0p:CHUNK],
                                             pst[:, :, i0p:CHUNK], Exp)
                        for idx in range(2):
                            jb = r0 + idx
                            i0 = max(0, (jb - 4 * ch) * 128)
                            nc.tensor.matmul(
                                po[:, i0:CHUNK], v_aug[:, inst, jb, :],
                                expt[:, idx, i0:CHUNK],
                                start=(jb == 0), stop=(jb == njb - 1))
                    # normalize: sums are replicated in po rows 64:128.
                    # 1/s via integer-magic seed + one Newton pass (max rel
                    # err 3.4e-3) -- ~2.5x cheaper than DVE InstReciprocal
                    # and pipelines as independent short ops.
                    y0 = sb.tile([DH, CHUNK], F32, tag="y0", bufs=2,
                                 name=f"y{bidx}{ch}{h}")
                    nc.vector.tensor_scalar(
                        y0[:].bitcast(I32), po[DH:2 * DH, :].bitcast(I32),
                        0x7EF127EA, -1,
                        mybir.AluOpType.subtract, mybir.AluOpType.mult)
                    u = sb.tile([DH, CHUNK], F32, tag="u", bufs=2,
                                name=f"u{bidx}{ch}{h}")
                    nc.vector.tensor_mul(u, po[0:DH, :], y0)
                    t = sb.tile([DH, CHUNK], F32, tag="tn", bufs=2,
                                name=f"t{bidx}{ch}{h}")
                    nc.vector.tensor_mul(t, po[DH:2 * DH, :], y0)
                    t2 = sb.tile([DH, CHUNK], F32, tag="t2", bufs=2,
                                 name=f"s{bidx}{ch}{h}")
                    nc.vector.tensor_scalar(
                        t2, t, -1.0, 2.0,
                        mybir.AluOpType.mult, mybir.AluOpType.add)
                    nc.vector.tensor_mul(ot[qr, :], u, t2)
            def p2_wo(bidx, ch):
                """Output projection for i-chunk ch of batch bidx. Output
                rows for adjacent db blocks are contiguous in outD, so two
                blocks share one DMA trigger (halves sync-queue issue time);
                the two PSUM->SBUF copies still split across DVE and ACT."""
                gcol = bidx * N + ch * CHUNK
                ot = ots.pop((bidx, ch))
                for dp in range(NCB // 2):
                    osb = sb.tile([128, 2, CHUNK], BF16, tag="osb", bufs=3,
                                  name=f"o{bidx}{ch}{dp}")
                    for half in range(2):
                        db = 2 * dp + half
                        ppr = ps.tile([128, CHUNK], F32, tag="sm", bufs=2,
                                      name=f"pj{bidx}{ch}{db}")
                        nc.tensor.matmul(ppr, wo[:, db, :], ot,
                                         start=True, stop=True)
                        if half == 0:
                            nc.vector.tensor_copy(osb[:, 0, :], ppr)
                        else:
                            nc.scalar.activation(
                                osb[:, 1, :], ppr,
                                mybir.ActivationFunctionType.Copy)
                    nc.sync.dma_start(
                        out=outD[dp * 256:(dp + 1) * 256,
                                 gcol:gcol + CHUNK].rearrange(
                            "(two p) n -> p two n", p=128),
                        in_=osb)

            for _ in range(reps):
                p1_chunk(0)
                p1_chunk(1)
                p2_attn(0, 0)
                p2_wo(0, 0)
                p1_chunk(2)
                dup_cossin_cols()
                p2_attn(0, 1)
                p2_wo(0, 1)
                p1_chunk(3)
                p2_attn(0, 2)
                p2_wo(0, 2)
                p1_chunk(4)
                p2_attn(0, 3)
                p2_wo(0, 3)
                p1_chunk(5)
                p2_attn(1, 0)
                p2_wo(1, 0)
                p1_chunk(6)
                p2_attn(1, 1)
                p2_wo(1, 1)
                p1_chunk(7)
                p2_attn(1, 2)
                p2_attn(1, 3)
                p2_wo(1, 2)
                p2_wo(1, 3)
    nc.compile()
    return nc


def _get_nc(reps=1):
    if reps not in _NC_CACHE:
        _NC_CACHE[reps] = build(reps)
    return _NC_CACHE[reps]


def make_in_maps(x, pos_emb, Wq, Wk, Wv, Wo):
    x = np.asarray(x, np.float32)
    pos_emb = np.asarray(pos_emb, np.float32)
    Wq = np.asarray(Wq, np.float32)
    Wk = np.asarray(Wk, np.float32)
    Wv = np.asarray(Wv, np.float32)
    Wo = np.asarray(Wo, np.float32)

    xT = np.ascontiguousarray(x.reshape(T, DIM).T).astype(NPBF16)  # [DIM, T]
    cosT1 = np.cos(pos_emb).T                            # [DH, N]
    cosT = np.ascontiguousarray(np.tile(cosT1, (2, 1))).astype(NPBF16)
    sinT = np.sin(pos_emb).T
    sins1 = np.concatenate([-sinT[0:32], sinT[32:64]], axis=0)
    sinsT = np.ascontiguousarray(np.tile(sins1, (2, 1))).astype(NPBF16)

    ident = np.eye(128, dtype=NPBF16)
    scale = np.float32(DH ** -0.5)
    jj = np.arange(128)[:, None]
    ii = np.arange(128)[None, :]
    masks = np.where(jj <= ii, 0.0, NEG).astype(NPBF16)
    ones_blk = np.ones((128, NINST, NJB, DH), NPBF16)

    in_maps = []
    for c in range(NCORES):
        cols = slice(c * 128, (c + 1) * 128)
        in_maps.append(dict(
            xT=xT,
            wq=(np.ascontiguousarray(Wq[:, cols]) * scale).astype(NPBF16),
            wk=np.ascontiguousarray(Wk[:, cols]).astype(NPBF16),
            wv=np.ascontiguousarray(Wv[:, cols]).astype(NPBF16),
            wo=np.ascontiguousarray(Wo[cols, :]).astype(NPBF16),
            cosT=cosT, sinsT=sinsT, identD=ident,
            masksD=masks, onesBlkD=ones_blk,
        ))
    return in_maps


def run(in_maps, trace=False, reps=1, **kw):
    nc = _get_nc(reps)
    return run_bass_kernel_spmd(nc, in_maps, list(range(NCORES)),
                                trace=trace, **kw)


def kernel(x, pos_emb, Wq, Wk, Wv, Wo, bo):
    in_maps = make_in_maps(x, pos_emb, Wq, Wk, Wv, Wo)
    res = run(in_maps)
    acc = np.zeros((DIM, T), np.float64)
    for c in range(NCORES):
        acc += np.asarray(res.results[c]["outT"], np.float32)
    out = acc.T.reshape(B, N, DIM) + np.asarray(bo, np.float32)[None, None, :]
    return out.astype(np.float32)
